# revision 1
# baseline (speedup 1.0000x reference)
"""Trainium2 Bass kernel for nn_AttentionControl (recurrent attention glimpse
network step, eval mode) — bf16 optimized version.

Contract: kernel(**inputs) takes the FULL unsharded inputs (B=512) and
returns the full [512, 256] f32 output. Pure data parallel across 8
NeuronCores (64 samples each). Host only does layout prep (pad, band,
transpose, dtype pack) and folds the constant crop+resize into the dense
weights.

Key differences vs the f32 baseline (HW steady-state 4.9us/invocation vs
~15us, single-shot sim 16us vs 22us; rel err 8.2e-4 vs 2e-2 budget):
  - whole window pipeline (banded image, gather, transposes, big matmuls,
    glimpse-fusion matmuls) runs in bf16 (fp32 PSUM accumulation); the
    location path stays exactly fp32 so the pixel rounding matches jax
    bit-for-bit.
  - X is sent pre-transposed + column-duplicated from the host, removing
    4 PE transposes + 8 DVE copies per iteration.
  - bands are 88 wide at stride 24 (1.375x overread vs 1.5x), and the
    whole half-window is ONE indirect gather (128 desc x 5.6KB), queues
    alternating between the two SWDGE queues.
  - the index chain is vectorized across loop iterations in the free dim
    ([128, 2n] ops): ~300ns/hop dependency latency on HW dominates small
    ops, so fewer/wider ops win; b_loc is folded into a K=1 matmul.
  - compaction drops the junk band cols into contiguous win2 (4x-mode DVE
    copies), then merged [128,128] PE transposes yield TWO matmul chunks
    each, 8 per PSUM bank, with both PSUM->SBUF copies on DVE: the scalar
    engine only ever runs Relu (activation-function-set switches cost
    ~1.3us on HW and are avoided entirely).
  - the final fusion computes g sample-major in one PSUM tile:
    g = relu(hgT^T Wgs + hlT^T Wls + 1^T biasrow), so no output transposes.
  - the body is software-pipelined [A A B B] with two pairs of lookahead
    so gather latency hides under the previous iterations' compute.
"""
import numpy as np

# ---------------- problem constants (hardcoded per contract) ----------------
B = 512
STATE = 512
S = 224
G = 256
HG = 128
HL = 128
TSB = 768
N_CORES = 8
NS = B // N_CORES            # samples per core = 64
PAD = 32                     # window pad (window = 64x64 around loc)
PADC_R = 48                  # right col pad so the last band (start 216) fits
PR = S + 2 * PAD             # padded rows = 288
BW = 88                      # band width
BSTRIDE = 24                 # band column stride
NBAND = 10                   # bands at column starts 0, 24, ..., 216
BANDE = PR * BW              # elements per band = 27648
SAMPE = NBAND * BANDE        # elements per sample = 221184
IMG2_ELEMS = NS * SAMPE + 4096  # +tail pad: last strip over-reads
IMG_ROWS = IMG2_ELEMS // 1024    # img declared [IMG_ROWS, 1024] so the src AP's
                                 # innermost run is wide (descriptor sizing)
NPIECE = 2                   # gather pieces (16 band rows each per half)
ROWS_PER_PIECE = 16
PIECE_RUN = ROWS_PER_PIECE * BW              # 1536 elems per partition piece
NTP = 16                     # merged transposes (each = 2 matmul chunks)
CPACK_W = 128 + 8 + 2 + 1 + 1 + 1 + 128 + 128      # f32 consts width
CPBF_W = 128 + 2 * G + 64 + G + 128                # bf16 consts width

_F32 = np.float32


def _resize_weight_mat(d, n=16):
    """jax.image.resize 'bilinear' (triangle kernel, antialias=True) weight
    matrix [d, n]; resized = w.T @ x @ w for a [d, d] input."""
    scale = _F32(n / d)
    inv_scale = _F32(1.0) / scale
    kernel_scale = np.maximum(inv_scale, _F32(1.0))
    sample_f = (np.arange(n, dtype=_F32) + _F32(0.5)) * inv_scale - _F32(0.5)
    x = np.abs(sample_f[None, :] - np.arange(d, dtype=_F32)[:, None]) / kernel_scale
    w = np.maximum(_F32(0), _F32(1) - np.abs(x)).astype(_F32)
    total = w.sum(axis=0, keepdims=True, dtype=_F32)
    w = np.where(np.abs(total) > 1000.0 * np.finfo(_F32).eps,
                 (w / np.where(total != 0, total, 1)).astype(_F32), 0.0).astype(_F32)
    keep = (sample_f >= -0.5) & (sample_f <= d - 0.5)
    return np.where(keep[None, :], w, 0.0).astype(_F32)


def _build_wwin(W_hg):
    """Fold crop-select + resize + W_hg into Wwin [4096, 128] acting on the
    flattened 64x64 window."""
    W = np.asarray(W_hg, dtype=np.float64)
    L = np.zeros((4096, TSB), dtype=np.float64)
    for i in range(16):
        for j in range(16):
            L[(24 + i) * 64 + (24 + j), i * 16 + j] = 1.0
    w32 = _resize_weight_mat(32).astype(np.float64)
    blk32 = np.einsum("ri,cj->rcij", w32, w32).reshape(32, 32, 256)
    for r in range(32):
        for c in range(32):
            L[(16 + r) * 64 + (16 + c), 256:512] = blk32[r, c]
    w64 = _resize_weight_mat(64).astype(np.float64)
    blk64 = np.einsum("ri,cj->rcij", w64, w64).reshape(64, 64, 256)
    for r in range(64):
        for c in range(64):
            L[r * 64 + c, 512:768] = blk64[r, c]
    return (L @ W).astype(_F32)  # [4096, 128]


# ---------------------------------------------------------------------------
# Bass program (built once, cached)
# ---------------------------------------------------------------------------
_CACHE = {}


def _build_nc(debug=False, loop_n=1, do_gather=True, do_tail=True, hw_loop=0,
              unroll=1, npiece=NPIECE, gather_mode="indirect", ablate=None):
    from contextlib import ExitStack, nullcontext
    import concourse.bass as bass
    import concourse.mybir as mybir
    import concourse.tile as tile
    from concourse import bacc

    dt = mybir.dt
    nc = bacc.Bacc("TRN2", target_bir_lowering=False, debug=False,
                   num_devices=N_CORES, num_swdge_queues=4)

    # ---- DRAM I/O ----
    img = nc.dram_tensor("img", [IMG_ROWS, 1024], dt.bfloat16, kind="ExternalInput")
    x_d = nc.dram_tensor("x", [128, 4 * 128], dt.float32, kind="ExternalInput")
    wwin_d = nc.dram_tensor("wwin", [128, NTP * 2 * HG], dt.bfloat16,
                            kind="ExternalInput")
    cpack_d = nc.dram_tensor("cpack", [128, CPACK_W], dt.float32, kind="ExternalInput")
    cpbf_d = nc.dram_tensor("cpbf", [128, CPBF_W], dt.bfloat16, kind="ExternalInput")
    out_d = nc.dram_tensor("out", [NS, G], dt.float32, kind="ExternalOutput")
    if debug:
        dbg_loc = nc.dram_tensor("dbg_loc", [128, 2], dt.float32, kind="ExternalOutput")
        dbg_idx = nc.dram_tensor("dbg_idx", [128, 1], dt.int32, kind="ExternalOutput")
        dbg_g = nc.dram_tensor("dbg_g", [128, NPIECE * PIECE_RUN], dt.float32,
                               kind="ExternalOutput")
        dbg_rhs = nc.dram_tensor("dbg_rhs", [128, NTP * 128], dt.float32,
                                 kind="ExternalOutput")
        dbg_hg = nc.dram_tensor("dbg_hg", [HG, NS], dt.float32, kind="ExternalOutput")

    F32 = dt.float32
    BF16 = dt.bfloat16
    Relu = mybir.ActivationFunctionType.Relu
    Copy = mybir.ActivationFunctionType.Copy
    Alu = mybir.AluOpType

    with tile.TileContext(nc) as tc, ExitStack() as ctx:
        const = ctx.enter_context(tc.tile_pool(name="const", bufs=1))
        work = ctx.enter_context(tc.tile_pool(name="work", bufs=4))
        small = ctx.enter_context(tc.tile_pool(name="small", bufs=6))
        psum_rot = ctx.enter_context(tc.tile_pool(name="psum_rot", bufs=2, space="PSUM"))
        psum_sm = ctx.enter_context(tc.tile_pool(name="psum_sm", bufs=3, space="PSUM"))
        psum_acc = ctx.enter_context(tc.tile_pool(name="psum_acc", bufs=2, space="PSUM"))

        # ---- load constants ----
        # x + cpack + cpbf first (needed by the loc chain); wwin after (only
        # needed by the main matmuls, overlaps with loc+gather).
        xt_sb = const.tile([128, 4, 128], F32, tag="xt")
        nc.sync.dma_start(xt_sb[:].rearrange("i k p -> i (k p)"), x_d.ap())
        cpack = const.tile([128, CPACK_W], F32, tag="cpack")
        nc.sync.dma_start(cpack[:], cpack_d.ap())
        cpbf = const.tile([128, CPBF_W], BF16, tag="cpbf")
        nc.sync.dma_start(cpbf[:], cpbf_d.ap())

        o = 0
        ident = cpack[:, o:o + 128]; o += 128
        wloc_sb = cpack[:, o:o + 8].rearrange("p (k u) -> p k u", k=4); o += 8
        bloc_sb = cpack[:, o:o + 2]; o += 2
        bhg_sb = cpack[:, o:o + 1]; o += 1
        bhl_sb = cpack[:, o:o + 1]; o += 1
        sampb_sb = cpack[:, o:o + 1]; o += 1
        whl_sb = cpack[0:2, o:o + 128]; o += 128
        onesf_sb = cpack[0:1, o:o + 128]; o += 128
        assert o == CPACK_W
        o = 0
        identb = cpbf[:, o:o + 128]; o += 128
        wgs_sb = cpbf[:, o:o + G]; o += G
        wls_sb = cpbf[:, o:o + G]; o += G
        ones_sb = cpbf[0:1, o:o + 64]; o += 64
        biasg_sb = cpbf[0:1, o:o + G]; o += G
        whlb_sb = cpbf[0:2, o:o + 128]; o += 128
        assert o == CPBF_W

        # ---- Wwin: [128 px, tp, h, hg] bf16, 4 chunked DMAs for overlap ----
        wwin_sb = const.tile([128, NTP, 2, HG], BF16, tag="wwin")
        wwin_flat = wwin_sb[:].rearrange("p t h f -> p (t h f)")
        for gi in range(4):
            nc.scalar.dma_start(wwin_flat[:, gi * 1024:(gi + 1) * 1024],
                                wwin_d.ap()[:, gi * 1024:(gi + 1) * 1024])

        # ---- body ----
        def indirect_gather_elem(out_ap, idx_ap, queue="qPoolDynamic"):
            eng = nc.gpsimd
            out_l = eng.lower_ap_dma(out_ap, for_indirect_dma=True)
            in_l = eng.lower_ap_dma(img.ap()[0:, :], for_indirect_dma=True)
            off_l = eng.lower_ap_dma(idx_ap)
            assert len(out_l) == 1 and len(in_l) == 1 and len(off_l) == 1
            in_l[0].dynamic_ap_info = mybir.DynamicAccessPatternInfo(
                c=0,
                actual_ap=out_ap.ap,
                indirect_dim_max_index=IMG2_ELEMS,
                offset_expr=[
                    mybir.DynamicAccessPatternOffsetExpr(
                        coef=1,
                        aff_expr=mybir.DynamicAccessPatternOffsetExprAffExpr(
                            kind="IndirectArgId", arg_id=1),
                    )
                ],
            )
            in_l.append(off_l[0])
            return eng.add_instruction(
                mybir.InstDMACopy(
                    name=nc.get_next_instruction_name(),
                    queue=queue,
                    mode="Copy",
                    ins=in_l,
                    outs=out_l,
                    oob_is_err=True,
                    cce_op=mybir.AluOpType.bypass,
                ))

        big = float(2.0 ** 23)
        qctr = [0]

        def phase_a_multi(n):
            """loc -> pixel round -> flat idx -> gather issue for n
            iterations, vectorized across the free dim: one [128, n]-wide
            DVE op per chain level instead of n ops, so both op count and
            the ~300ns dependency-hop latency amortize. The hl branch is
            emitted in phase_b so the PE queue never blocks on this chain."""
            T = [dict() for _ in range(n)]
            # loc matmuls; b_loc folded in as a K=1 matmul with a ones row
            plocm = psum_sm.tile([128, 2 * n], F32, tag="sm", name="plocm")
            for j in range(n):
                sl = slice(2 * j, 2 * j + 2)
                for k in range(4):
                    nc.tensor.matmul(plocm[:, sl], xt_sb[:, k, :],
                                     wloc_sb[:, k, :],
                                     start=(k == 0), stop=False)
                nc.tensor.matmul(plocm[:, sl], onesf_sb[:], bloc_sb[0:1, :],
                                 start=False, stop=True)
            # loc = hard_tanh(ploc) (bias already in)
            locm = small.tile([128, 2 * n], F32, tag="loc", name="locm")
            nc.vector.tensor_scalar(locm[:], plocm[:], 1.0, -1.0,
                                    op0=Alu.min, op1=Alu.max)
            for j in range(n):
                T[j]["loc_sb"] = locm[:, 2 * j:2 * j + 2]
            # tco = lr + 2^23 where lr = RNE(112*loc + 112), exact ints
            tcom = small.tile([128, 2 * n], F32, tag="tco", name="tcom")
            nc.vector.tensor_scalar(tcom[:], locm[:], 112.0, 112.0 + big,
                                    op0=Alu.mult, op1=Alu.add)
            lrm = small.tile([128, 2 * n], F32, tag="lr", name="lrm")
            nc.vector.tensor_scalar_sub(lrm[:], tcom[:], big)
            # band+1 = RNE(l1/BSTRIDE + 0.51); l1*(1/24) has ~6e-8 rel
            # error against the 0.01 margins to the 0.5/1.5 round boundaries
            ubm = small.tile([128, n], F32, tag="ub", name="ubm")
            nc.vector.tensor_scalar(ubm[:], lrm[:, 1::2], 1.0 / BSTRIDE,
                                    0.51, op0=Alu.mult, op1=Alu.add)
            bandm = small.tile([128, n], F32, tag="band", name="bandm")
            nc.vector.tensor_scalar(bandm[:], ubm[:], big, big,
                                    op0=Alu.add, op1=Alu.subtract)
            # flat base A = sampb' + (band+1)*(BANDE-BSTRIDE) + l0*BW + l1
            # (the -1 band shift is folded into sampb'); everything stays
            # exactly representable in f32 (< 2^24).
            am = small.tile([128, n], F32, tag="abase", name="am")
            nc.vector.tensor_scalar(am[:], bandm[:], float(BANDE - BSTRIDE),
                                    sampb_sb[:], op0=Alu.mult, op1=Alu.add)
            t1m = small.tile([128, n], F32, tag="vtmp", name="t1m")
            nc.vector.tensor_scalar_mul(t1m[:], lrm[:, 0::2], float(BW))
            vm = small.tile([128, n], F32, tag="vtmp2", name="vm")
            nc.vector.tensor_tensor(vm[:], t1m[:], lrm[:, 1::2], op=Alu.add)
            # idx = int32(a + v): exact integer, converted on write
            idxm = small.tile([128, n], dt.int32, tag="idx", name="idxm")
            nc.vector.tensor_tensor(idxm[:], am[:], vm[:], op=Alu.add)
            # gather: one strip of 32 full-width band rows per partition
            # (p = 64h + s reads window rows 32h..32h+31, contiguous);
            # alternate SWDGE queues so back-to-back gathers overlap
            for j in range(n):
                if ablate == "loconly":
                    T[j]["gbuf"] = None
                    continue
                gbuf = work.tile([128, 2 * ROWS_PER_PIECE, BW], BF16,
                                 tag="gbuf", bufs=8, name=f"gbuf{j}")
                if do_gather:
                    if gather_mode == "indirect":
                        qi = qctr[0] % 4
                        qctr[0] += 1
                        indirect_gather_elem(
                            gbuf[:].rearrange("p q c -> p (q c)"),
                            idxm[:, j:j + 1],
                            queue=f"qPoolDynamic{qi if qi else ''}")
                    else:  # same-size static DMA (ablation only; wrong data)
                        nc.sync.dma_start(
                            gbuf[:].rearrange("p q c -> p (q c)"),
                            img.ap()[0:384, :].rearrange(
                                "(p k) x -> p (k x)", k=3))
                T[j]["gbuf"] = gbuf
                T[j]["idx_sb"] = idxm[:, j:j + 1]
            return T

        def phase_a():
            return phase_a_multi(1)[0]

        def phase_b(t):
            """Window compute: compact (drop junk cols 64..95) into contiguous
            win2 (4x-mode DVE copy), merged [128,128] transposes 4 per PSUM
            group, copy each group to SBUF (alt DVE/scalar), that group's 8
            matmuls; then the fusion tail.
            win2[64h+s, 2tp+rho, c] = window row 32h+2tp+rho col c;
            transpose tp reads win2[:, 2tp:2tp+2, :] (contiguous 128):
            out[rho*64+c, 64h+s] -> cols 0:64 = chunk (h=0, tp), cols 64:128
            = chunk (h=1, tp) = window rows 32h+2tp, 32h+2tp+1."""
            gbuf = t["gbuf"]
            win2 = work.tile([128, 2 * NTP, 64], BF16, tag="win2")
            rhs_sb = work.tile([128, NTP, 128], BF16, tag="rhs")
            phg = psum_acc.tile([HG, NS], F32, tag="acc", name="phg")

            def do_compact():
                nc.vector.tensor_copy(win2[:], gbuf[:, :, 0:64])

            rot = [None] * 2

            def do_group_transposes(g):
                # 8 transposes into one full PSUM bank (2KB = [128,8,128]bf16)
                rot[g] = psum_rot.tile([128, 8, 128], BF16, tag="rot",
                                       name=f"rot{g}")
                for i in range(8):
                    tp = 8 * g + i
                    nc.tensor.transpose(
                        rot[g][:, i, :],
                        win2[:, 2 * tp:2 * tp + 2, :].rearrange(
                            "p r c -> p (r c)"),
                        identb[:])

            def do_group_copy(g):
                nc.vector.tensor_copy(rhs_sb[:, 8 * g:8 * g + 8, :], rot[g][:])

            def do_group_matmuls(g, last):
                for i in range(8):
                    tp = 8 * g + i
                    for h in range(2):
                        nc.tensor.matmul(
                            phg[:], wwin_sb[:, tp, h, :],
                            rhs_sb[:, tp, 64 * h:64 * h + 64],
                            start=(g == 0 and i == 0 and h == 0),
                            stop=(last and i == 7 and h == 1))

            do_compact()
            do_group_transposes(0)
            do_group_copy(0)
            do_group_transposes(1)
            # hl branch (emitted here so the PE reaches it only after the
            # transposes, when locT's copy has long landed)
            locb = small.tile([128, 2], BF16, tag="locb")
            nc.vector.tensor_copy(locb[:], t["loc_sb"][:])
            plocT = psum_sm.tile([2, 128], BF16, tag="sm", name="plocT")
            nc.tensor.transpose(plocT[:], locb[:], identb[:])
            locT_sb = small.tile([2, 128], BF16, tag="locT")
            nc.vector.tensor_copy(locT_sb[:], plocT[:])
            phl = psum_sm.tile([HL, NS], F32, tag="sm", name="phl")
            nc.tensor.matmul(phl[:], whlb_sb[:], locT_sb[:, 0:NS],
                             start=True, stop=True)
            hlT_sb = work.tile([HL, NS], BF16, tag="hlT")
            nc.scalar.activation(hlT_sb[:], phl[:], Relu, bias=bhl_sb[:])
            do_group_copy(1)
            for g in range(2):
                do_group_matmuls(g, last=(g == 1))

            # g sample-major: relu(hgT^T Wgs + hlT^T Wls + 1^T bias).
            # The hl + bias terms don't depend on hgT, so they accumulate
            # right after the window matmuls; only the hgT term remains in
            # the post-relu tail (shortens the exposed drain chain by two
            # dependency hops).
            pg = psum_acc.tile([NS, G], F32, tag="acc", name="pg")
            nc.tensor.matmul(pg[:], hlT_sb[:], wls_sb[:], start=True, stop=False)
            nc.tensor.matmul(pg[:], ones_sb[:], biasg_sb[:], start=False,
                             stop=False)
            hgT_sb = work.tile([HG, NS], BF16, tag="hgT")
            nc.scalar.activation(hgT_sb[:], phg[:], Relu, bias=bhg_sb[:])
            nc.tensor.matmul(pg[:], hgT_sb[:], wgs_sb[:], start=False, stop=True)
            g_sb = work.tile([NS, G], F32, tag="g")
            nc.scalar.activation(g_sb[:], pg[:], Relu)

            nc.sync.dma_start(out_d.ap(), g_sb[:])
            if debug:
                nc.sync.dma_start(dbg_loc.ap(), t["loc_sb"][:])
                nc.sync.dma_start(dbg_idx.ap(), t["idx_sb"][:])
                dgf = work.tile([128, NPIECE * PIECE_RUN], F32, tag="dbgg")
                nc.vector.tensor_copy(dgf[:],
                                      gbuf[:].rearrange("p q c -> p (q c)"))
                nc.sync.dma_start(dbg_g.ap(), dgf[:])
                drf = work.tile([128, NTP * 128], F32, tag="dbgr")
                nc.vector.tensor_copy(drf[:],
                                      rhs_sb[:].rearrange("p t f -> p (t f)"))
                nc.sync.dma_start(dbg_rhs.ap(), drf[:])
                dhf = work.tile([HG, NS], F32, tag="dbgh")
                nc.vector.tensor_copy(dhf[:], hgT_sb[:])
                nc.sync.dma_start(dbg_hg.ap(), dhf[:])

        # ---- body: software-pipelined [A(i+1) before B(i)] so the gather
        # latency of the next iteration hides under the current compute ----
        if ablate in ("gatheronly", "gather2q"):
            t0 = phase_a()   # one real A outside the loop supplies idx
            idx0 = t0["idx_sb"]
            with tc.For_i(0, hw_loop or 1, 1):
                for _it in range(loop_n * unroll):
                    gb = work.tile([128, 2 * ROWS_PER_PIECE, BW], BF16,
                                   tag="gbufab", name=f"gab{_it}")
                    q = ("qPoolDynamic1" if (ablate == "gather2q" and _it % 2)
                         else "qPoolDynamic")
                    indirect_gather_elem(
                        gb[:].rearrange("p q c -> p (q c)"), idx0[:], queue=q)
        elif ablate == "empty":
            with tc.For_i(0, hw_loop or 1, 1):
                for _it in range(loop_n * unroll):
                    z = small.tile([128, 1], F32, tag="ztiny", name=f"z{_it}")
                    nc.vector.tensor_copy(z[:], sampb_sb[:])
        else:
            # A-phases vectorized in groups (quads when the unroll count
            # allows), one group of lookahead: [A4, A4, B*4, A4, B*4, ...]
            loop_cm = tc.For_i(0, hw_loop, 1) if hw_loop else nullcontext()
            with loop_cm:
                k = loop_n * unroll
                gsz = 2 if k % 2 == 0 else 1
                groups = []
                for _p in range(k // gsz):
                    groups.append(phase_a_multi(gsz))
                    if do_tail and len(groups) >= 3:
                        for t in groups.pop(0):
                            phase_b(t)
                if do_tail:
                    for gr in groups:
                        for t in gr:
                            phase_b(t)

    nc.compile()
    return nc


def _host_prep(inputs):
    """Build the per-core in_maps (pure layout transforms of the inputs)."""
    import ml_dtypes
    BF = ml_dtypes.bfloat16
    X = np.ascontiguousarray(np.asarray(inputs["output"], dtype=_F32))
    img = np.asarray(inputs["inputs"], dtype=_F32)[..., 0]
    W_loc = np.asarray(inputs["W_loc"], dtype=_F32)
    b_loc = np.asarray(inputs["b_loc"], dtype=_F32)
    W_hl = np.asarray(inputs["W_hl"], dtype=_F32)
    b_hl = np.asarray(inputs["b_hl"], dtype=_F32)
    W_gs = np.asarray(inputs["W_gs"], dtype=_F32)
    b_gs = np.asarray(inputs["b_gs"], dtype=_F32)
    W_ls = np.asarray(inputs["W_ls"], dtype=_F32)
    b_ls = np.asarray(inputs["b_ls"], dtype=_F32)

    wwin = _build_wwin(inputs["W_hg"])  # [4096, 128] f32
    # chunk (tp, h) = window rows (32h+2tp, 32h+2tp+1) -> [128 px, 128 hg];
    # wwin_r[p, tp, h, :] with px p = rho*64+c
    wwin_r = wwin.reshape(2, 16, 2, 64, HG)       # [h, tp, rho, c, hg]
    wwin_r = wwin_r.transpose(2, 3, 1, 0, 4)      # [rho, c, tp, h, hg]
    wwin_r = wwin_r.reshape(128, NTP * 2 * HG).astype(BF)

    # padded [B, 288, 304] -> bands [B, 8, 288, 96] bf16
    padded = np.pad(img, ((0, 0), (PAD, PAD), (PAD, PADC_R))).astype(BF)
    bands = np.stack([padded[:, :, BSTRIDE * k:BSTRIDE * k + BW]
                      for k in range(NBAND)], axis=1)

    p = np.arange(128)
    sampbase = ((p % 64) * SAMPE + (p // 64) * (2 * ROWS_PER_PIECE * BW)
                - (BANDE - BSTRIDE)).astype(_F32).reshape(128, 1)
    ident = np.eye(128, dtype=_F32)
    bloc_b = np.broadcast_to(b_loc, (128, 2)).astype(_F32)
    bhg = np.asarray(inputs["b_hg"], dtype=_F32).reshape(HG, 1)
    bhl = b_hl.reshape(HL, 1)
    whl_pad = np.zeros((128, 128), _F32)
    whl_pad[0:2, :] = W_hl
    cpack = np.concatenate([
        ident,
        W_loc.reshape(4, 128, 2).transpose(1, 0, 2).reshape(128, 8),
        bloc_b, bhg, bhl, sampbase, whl_pad,
        np.ones((128, 128), _F32),
    ], axis=1).astype(_F32)
    assert cpack.shape == (128, CPACK_W)

    onesrow = np.zeros((128, 64), _F32); onesrow[0, :] = 1.0
    biasrow = np.zeros((128, G), _F32); biasrow[0, :] = (b_gs + b_ls)
    cpbf = np.concatenate([
        ident, W_gs, W_ls, onesrow, biasrow, whl_pad,
    ], axis=1).astype(BF)
    assert cpbf.shape == (128, CPBF_W)

    # xdupT [128, 512]: row i, col (128k+p) -> X[p%64, 128k+i]
    in_maps = []
    for c in range(N_CORES):
        sl = slice(c * NS, (c + 1) * NS)
        xc = X[sl].T.reshape(4, 128, NS)                    # [k, i, s]
        xdup = np.concatenate([xc, xc], axis=2)             # [k, i, p]
        xdup = xdup.transpose(1, 0, 2).reshape(128, 512)    # [i, (k p)]
        in_maps.append({
            "img": np.concatenate(
                [bands[sl].reshape(-1),
                 np.zeros(4096, BF)]).reshape(-1, 1024),
            "x": np.ascontiguousarray(xdup.astype(_F32)),
            "wwin": wwin_r,
            "cpack": cpack,
            "cpbf": cpbf,
        })
    return in_maps


def kernel(**inputs) -> np.ndarray:
    from concourse.bass_utils import run_bass_kernel_spmd

    if "nc" not in _CACHE:
        _CACHE["nc"] = _build_nc()
    nc = _CACHE["nc"]
    in_maps = _host_prep(inputs)
    res = run_bass_kernel_spmd(nc, in_maps, core_ids=list(range(N_CORES)))
    out = np.concatenate([res.results[c]["out"] for c in range(N_CORES)], axis=0)
    return out.astype(np.float32)

